# revision 18
# baseline (speedup 1.0000x reference)
"""Trainium2 Bass kernel for the L2-normalized attention module.

Reference computation (per batch b):
    qkv = x @ w_qkv.T                        # [n, 3*dim]
    q,k,v per head h (d=64)                  # [n, d]
    dots = q @ k.T                           # [n, n]
    attn = dots / max(||dots_row||_2, eps) * g + b
    out_h = attn @ v                         # [n, d]
    final = concat_h(out_h) @ w_out.T + b_out

Key algebraic factorization: the score "nonlinearity" is only a per-row
scale r_i = 1/max(||dots_i||, eps), and ||dots_i||^2 = q_i^T (k^T k) q_i.
With W = k^T v, G = k^T k:
    out_h^T[:, i] = r_i * (W^T q_i),   r_i = rsqrt(q_i^T G q_i)
so the n x n score matrix never exists.  Additionally W^T is fused into
the output projection on device (F = W^T-blockdiag @ wo^T), so the final
projection consumes rinv-scaled q directly:
    final^T = F^T (q * rinv_broadcast)

Schedule (single PE stream, emission order = execution order):
  stage 1: kv projection + incremental Gram accumulation, chunk by chunk
           (overlapped with the x DMA fill; junk warmup matmuls run
           during the fill to open the HAM clock-gate early).
  stage 2: per 512-column chunk, software-pipelined so the PE never
           idles: q projection of chunk c+1 covers the
           norm2 -> sqrt -> reciprocal -> scale chain of chunk c, and the
           output projection of chunk c covers the next chain.

Sharding: 8 cores = 2 batches x 4 head-groups (4 heads each).  The host
sums the 4 partial w_out products per batch and adds b_out.  norm_g and
the 2^-4 rinv compensation are folded into w_out on the host; norm_b
(zero in practice) is handled by an exact host-side rank-1 correction.
"""

import numpy as np

from concourse import bacc
from concourse import masks
import concourse.mybir as mybir
import concourse.tile as tile
from concourse.bass_utils import run_bass_kernel_spmd

# Problem shape (hardcoded per contract)
B, N, DIM, H, D = 2, 2048, 1024, 16, 64
NCORES = 8
HPC = H // 4            # 4 heads per core
CH = 512                # sequence chunk (matmul moving free dim)
NCH = N // CH           # 4
KO = DIM // 128         # 8 contraction tiles for the projections
P = 128

SG = 2.0 ** -8          # G scale so q*(G q) products stay in fp16 range
SW = 2.0 ** -4          # folded into w_out on host; compensates rinv*2^4

F32 = mybir.dt.float32
F16 = mybir.dt.float16
MULT = mybir.AluOpType.mult
ADD = mybir.AluOpType.add
AFT = mybir.ActivationFunctionType

N_WARM = 64             # junk matmuls bridging the DMA fill


def _build_bass():
    nc = bacc.Bacc("TRN2", target_bir_lowering=False, debug=False)

    # eps^2 const for the sqrt bias (mirrors Bass's const registration)
    _eps_t = nc.alloc_sbuf_tensor("const-float32-eps2q", [128, 1], F32)
    nc.gpsimd.memset(_eps_t.ap(), 2.5e-25)
    nc.const_aps.aps[(F32, 2.5e-25)] = _eps_t.ap()

    x_d = nc.dram_tensor("xt", [NCH, P, KO, CH], F16, kind="ExternalInput").ap()
    wq_d = nc.dram_tensor("wq", [P, KO, 256], F16, kind="ExternalInput").ap()
    wkv_d = nc.dram_tensor("wkv", [P, KO, 512], F16, kind="ExternalInput").ap()
    wo_d = nc.dram_tensor("wo", [P, 2, 1024], F16, kind="ExternalInput").ap()
    out_d = nc.dram_tensor("outT", [NCH, 2, P, 4, CH], F16,
                           kind="ExternalOutput").ap()

    with tile.TileContext(nc) as tc:
        with (
            tc.tile_pool(name="w", bufs=1) as wpool,
            tc.tile_pool(name="big", bufs=1) as bigpool,
            tc.tile_pool(name="small", bufs=4) as smallpool,
            tc.tile_pool(name="stage", bufs=4) as stagepool,
            tc.tile_pool(name="psA", bufs=2, space="PSUM") as psA,
            tc.tile_pool(name="psQ", bufs=2, space="PSUM") as psQ,
            tc.tile_pool(name="psT", bufs=2, space="PSUM") as psT,
            tc.tile_pool(name="psG", bufs=2, space="PSUM") as psG,
        ):
            # ---- on-chip constants (no DMA needed) ----
            warm = wpool.tile([P, 128], F16, name="warm")
            nc.gpsimd.memset(warm, 0.007812)
            bo = wpool.tile([P, 128], F16, name="bo_sb")
            nc.gpsimd.memset(bo, 0.0)
            nc.gpsimd.memset(bo[0:64, 0:64], 1.0)
            nc.gpsimd.memset(bo[64:128, 64:128], 1.0)
            ident = wpool.tile([P, 128], F32, name="ident")
            masks.make_identity(nc, ident)
            gwG_all = wpool.tile([P, 2, 128], F16, name="gwG_all")
            nc.gpsimd.memset(gwG_all, 0.0)
            gwWT_all = wpool.tile([P, 2, 128], F16, name="gwWT_all")
            nc.gpsimd.memset(gwWT_all, 0.0)

            # ---- warmup: opens the HAM clock-gate (K=8/8) during the
            # DMA fill; results are discarded.
            warm_sb = smallpool.tile([P, 4], F32, name="warm_sb")
            ps_warm = psA.tile([P, CH], F32, tag="psA", name="ps_warm")
            for _ in range(N_WARM):
                nc.tensor.matmul(ps_warm[:, 0:128], lhsT=warm, rhs=warm,
                                 start=True, stop=True)
            nc.vector.tensor_copy(warm_sb, ps_warm[:, 0:4])

            # ---- input DMAs, ordered by first use; wkv/x0 split in ko
            # halves so the first kv accumulation group starts ~2.6us
            # earlier than a monolithic transfer would allow.
            wkv = wpool.tile([P, KO, 512], F16, name="wkv_sb")
            x_all = wpool.tile([P, NCH, KO, CH], F16, name="x_all")
            for qq in range(2):
                ks = slice(4 * qq, 4 * qq + 4)
                nc.sync.dma_start(wkv[:, ks, :], wkv_d[:, ks, :])
                nc.sync.dma_start(x_all[:, 0, ks, :], x_d[0, :, ks, :])
            for cc in range(1, NCH):
                nc.sync.dma_start(x_all[:, cc, :, :], x_d[cc])
            wq = wpool.tile([P, KO, 256], F16, name="wq_sb")
            nc.sync.dma_start(wq, wq_d)
            wo = wpool.tile([P, 2, 1024], F16, name="wo_sb")
            nc.sync.dma_start(wo, wo_d)

            # ---- persistent tiles ----
            qT = bigpool.tile([P, 2, N], F16, name="qT_sb")        # [pair, n]
            qs = bigpool.tile([P, 2, N], F16, name="qs_sb")        # rinv-scaled q
            kv = bigpool.tile([P, 16, CH], F16, name="kv_sb")      # [n-tile, per-pair [ke|ko|ve|vo]]
            F_sb = bigpool.tile([P, 2, 1024], F16, name="F_sb")    # fused W^T @ woT
            gwacc = wpool.tile([P, 2, 256], F32, name="gwacc")

            # ---- stage 1: kv projection + incremental Gram ----
            # kv[j, c]  = sum_k x^T[k, j] wkv[k, c]   (c = per-pair [k|v])
            # [G|W] per pair accumulates per kv tile as it is produced.
            # Gram matmuls run one kv tile behind the projection so the PE
            # never waits on the PSUM->SBUF copy of the tile it grams.
            gw_part = [
                psG.tile([P, 256], F32, tag="psG", name=f"gw_part{p}")
                for p in range(2)
            ]

            def gram(jt):
                for p in range(2):
                    blk = kv[:, jt, p * 256:(p + 1) * 256]
                    nc.tensor.matmul(
                        gw_part[p], lhsT=blk[:, 0:128], rhs=blk,
                        start=(jt == 0), stop=(jt == 15),
                    )

            for c in range(NCH):
                for nt in range(4):
                    jt = c * 4 + nt
                    ps_kv = psA.tile([P, CH], F32, tag="psA", name="ps_kv")
                    for ko in range(KO):
                        nc.tensor.matmul(
                            ps_kv,
                            lhsT=(x_all[:, c, ko, nt * 128:(nt + 1) * 128]),
                            rhs=(wkv[:, ko, :]),
                            start=(ko == 0),
                            stop=(ko == KO - 1),
                        )
                    if nt % 2 == 0:
                        nc.scalar.copy(kv[:, jt, :], ps_kv)
                    else:
                        nc.vector.tensor_copy(kv[:, jt, :], ps_kv)
                    if jt > 0:
                        gram(jt - 1)
            gram(15)
            for p in range(2):
                nc.vector.tensor_copy(gwacc[:, p, :], gw_part[p])

            # Gram finalize: block-diagonal G (scaled by SG, two heads
            # stacked on K=128) on DVE/ACT; these overlap the q0 matmuls.
            for p in range(2):
                nc.vector.tensor_scalar_mul(gwG_all[0:64, p, 0:64],
                                            gwacc[0:64, p, 0:64], SG)
                nc.scalar.activation(gwG_all[64:128, p, 64:128],
                                     gwacc[64:128, p, 64:128],
                                     AFT.Copy, bias=0.0, scale=SG)

            # ---- stage 2: q projection pipelined with the norm chain
            # and the fused output projection.
            def q_proj(c):
                cs = slice(c * CH, (c + 1) * CH)
                for mt in range(2):
                    ps_q = psQ.tile([P, CH], F32, tag="psQ", name="ps_q")
                    for ko in range(KO):
                        nc.tensor.matmul(
                            ps_q,
                            lhsT=(wq[:, ko, mt * 128:(mt + 1) * 128]),
                            rhs=(x_all[:, c, ko, :]),
                            start=(ko == 0),
                            stop=(ko == KO - 1),
                        )
                    if mt == 0:
                        nc.vector.tensor_copy(qT[:, mt, cs], ps_q)
                    else:
                        nc.scalar.copy(qT[:, mt, cs], ps_q)

            def t_mm(c):
                # ps_t = G q (PE), prods = (G q) * q elementwise (DVE)
                cs = slice(c * CH, (c + 1) * CH)
                out = {}
                for p in range(2):
                    ps_t = psT.tile([P, CH], F32, tag="psT", name="ps_t")
                    nc.tensor.matmul(ps_t, lhsT=(gwG_all[:, p, :]),
                                     rhs=(qT[:, p, cs]), start=True, stop=True)
                    pr = stagepool.tile([P, CH], F16, name="prod",
                                        tag="prod", bufs=4)
                    nc.vector.tensor_tensor(pr, ps_t, qT[:, p, cs], MULT)
                    out[p] = pr
                return out

            def rep_mm(c, prods):
                # norm2 broadcast per head (fp16 block-ones matmul), then
                # rinv = 1/sqrt(norm2+eps') via ACT Sqrt + DVE reciprocal.
                cs = slice(c * CH, (c + 1) * CH)
                for p in range(2):
                    ps_rep = psT.tile([P, CH], F32, tag="psT", name="ps_rep")
                    nc.tensor.matmul(ps_rep, lhsT=(bo), rhs=(prods[p]),
                                     start=True, stop=True)
                    sq = stagepool.tile([P, CH], F32, name="sqr",
                                        tag="sqr", bufs=4)
                    nc.scalar.activation(sq, ps_rep, AFT.Sqrt, bias=2.5e-25)
                    ri = stagepool.tile([P, CH], F32, name="rinv",
                                        tag="rinv", bufs=4)
                    nc.vector.reciprocal_approx_fast(out=ri, in_=sq)
                    nc.gpsimd.tensor_tensor(qs[:, p, cs], qT[:, p, cs],
                                            ri, MULT)

            def c_proj(c, mts, fine=False):
                # final^T partial: F^T (q*rinv), batched stores per kk-half
                # (per-mt stores for the last chunk so the tail is short)
                cs = slice(c * CH, (c + 1) * CH)
                for kk, st in mts:
                    for mtl in range(4):
                        mt = kk * 4 + mtl
                        ps_f = psA.tile([P, CH], F32, tag="psA", name="ps_f")
                        for p in range(2):
                            nc.tensor.matmul(
                                ps_f,
                                lhsT=(F_sb[:, p, mt * 128:(mt + 1) * 128]),
                                rhs=(qs[:, p, cs]),
                                start=(p == 0),
                                stop=(p == 1),
                            )
                        if mtl % 2 == 0:
                            nc.vector.tensor_copy(st[:, mtl, :], ps_f)
                        else:
                            nc.scalar.copy(st[:, mtl, :], ps_f)
                        if fine:
                            nc.sync.dma_start(out_d[c, kk, :, mtl, :],
                                              st[:, mtl, :])
                    if not fine:
                        nc.sync.dma_start(out_d[c, kk], st)

            def st_tiles():
                return [(kk, stagepool.tile([P, 4, CH], F16, name="st",
                                            tag="st", bufs=4))
                        for kk in range(2)]

            # q0, then W^T transposes + F build (covered by q0), then the
            # per-chunk pipeline.
            q_proj(0)

            for p in range(2):
                ps_wt = psG.tile([P, 256], F32, tag="psG", name="ps_wt")
                nc.tensor.transpose(ps_wt[:, 0:128], gwacc[:, p, 128:256],
                                    ident)
                nc.vector.tensor_copy(gwWT_all[0:64, p, 0:64],
                                      ps_wt[0:64, 0:64])
                nc.vector.tensor_copy(gwWT_all[64:128, p, 64:128],
                                      ps_wt[64:128, 64:128])

            t0 = t_mm(0)
            for p in range(2):
                for hf in range(2):
                    ps_F = psA.tile([P, CH], F32, tag="psA", name="ps_F")
                    nc.tensor.matmul(
                        ps_F, lhsT=(gwWT_all[:, p, :]),
                        rhs=(wo[:, p, hf * 512:(hf + 1) * 512]),
                        start=True, stop=True)
                    if hf == 0:
                        nc.scalar.copy(F_sb[:, p, 0:512], ps_F)
                    else:
                        nc.vector.tensor_copy(F_sb[:, p, 512:1024], ps_F)
            rep_mm(0, t0)

            # Chunk 0's output projection is deferred to the very end: its
            # qs is ready early, so it gives the PE chain-free work that
            # covers chunk 3's norm chain (no tail stall).
            q_proj(1)
            t1 = t_mm(1)
            rep_mm(1, t1)

            q_proj(2)
            st1 = st_tiles()
            c_proj(1, st1[:1])
            t2 = t_mm(2)
            c_proj(1, st1[1:])
            rep_mm(2, t2)

            q_proj(3)
            st2 = st_tiles()
            c_proj(2, st2[:1])
            t3 = t_mm(3)
            c_proj(2, st2[1:])
            rep_mm(3, t3)

            c_proj(0, st_tiles())
            c_proj(3, st_tiles(), fine=True)

    nc.compile()
    return nc


_NC_CACHE = None


def _get_nc():
    global _NC_CACHE
    if _NC_CACHE is None:
        _NC_CACHE = _build_bass()
    return _NC_CACHE


def _build_in_maps(x, w_qkv, w_out_g):
    """Per-core device inputs (shared NEFF, different shards)."""
    in_maps = []
    for core in range(NCORES):
        bi = core // 4
        hg = core % 4
        # x^T tiled chunk-major [nch, p, ko, ch]
        xt0 = x[bi].T.reshape(KO, P, N).transpose(1, 0, 2)
        xt = np.ascontiguousarray(
            np.stack([xt0[:, :, cc * CH:(cc + 1) * CH] for cc in range(NCH)]))
        # q rows of this head group, transposed -> [dim, 256] -> [p, ko, 256]
        rows_q = slice(hg * 256, hg * 256 + 256)
        wq = np.ascontiguousarray(
            w_qkv[rows_q].T.reshape(KO, P, 256).transpose(1, 0, 2))
        # per-head-pair [k_even | k_odd | v_even | v_odd] blocks
        blocks = []
        for pp in range(2):
            he = hg * HPC + 2 * pp
            ho = he + 1
            blocks.append(w_qkv[DIM + he * D: DIM + (he + 1) * D])
            blocks.append(w_qkv[DIM + ho * D: DIM + (ho + 1) * D])
            blocks.append(w_qkv[2 * DIM + he * D: 2 * DIM + (he + 1) * D])
            blocks.append(w_qkv[2 * DIM + ho * D: 2 * DIM + (ho + 1) * D])
        wkv_local = np.concatenate(blocks, axis=0)  # [512, dim]
        wkv = np.ascontiguousarray(
            wkv_local.T.reshape(KO, P, 512).transpose(1, 0, 2))
        # w_out columns for this head group (norm_g and SW folded), transposed
        wo_local = w_out_g[:, hg * 256:(hg + 1) * 256]  # [1024, 256]
        wo = np.ascontiguousarray(
            wo_local.T.reshape(2, P, 1024).transpose(1, 0, 2))
        in_maps.append({
            "xt": xt.astype(np.float16), "wq": wq.astype(np.float16),
            "wkv": wkv.astype(np.float16), "wo": wo.astype(np.float16),
        })
    return in_maps


def kernel(x, w_qkv, w_out, b_out, norm_g, norm_b):
    x = np.ascontiguousarray(np.asarray(x, dtype=np.float32))
    w_qkv = np.asarray(w_qkv, dtype=np.float32)
    w_out = np.asarray(w_out, dtype=np.float32)
    b_out = np.asarray(b_out, dtype=np.float32)
    g = np.asarray(norm_g, dtype=np.float32).reshape(H)
    bb = np.asarray(norm_b, dtype=np.float32).reshape(H)

    # Fold norm_g and the 2^-4 rinv compensation into w_out columns.
    w_out_g = w_out.copy()
    for h in range(H):
        w_out_g[:, h * D:(h + 1) * D] *= g[h] * SW

    in_maps = _build_in_maps(x, w_qkv, w_out_g)

    nc = _get_nc()
    res = None
    last_exc = None
    for _attempt in range(3):
        try:
            res = run_bass_kernel_spmd(nc, in_maps, core_ids=list(range(NCORES)))
            break
        except Exception as e:  # transient NRT_EXEC_UNIT_UNRECOVERABLE etc.
            last_exc = e
            import time as _time
            _time.sleep(5)
    if res is None:
        raise last_exc

    out = np.zeros((B, N, DIM), np.float32)
    for core in range(NCORES):
        bi = core // 4
        buf = res.results[core]["outT"]  # [NCH, 2, P, 4, CH]
        partial = buf.transpose(1, 3, 2, 0, 4).reshape(DIM, N).astype(np.float32)
        out[bi] += partial.T
    out += b_out[None, None, :]

    # Exact rank-1 correction for norm_b (zero in practice).
    if np.any(bb != 0.0):
        for bi in range(B):
            corr = np.zeros(DIM, np.float64)
            for h in range(H):
                wv = w_qkv[2 * DIM + h * D: 2 * DIM + (h + 1) * D]  # [d, dim]
                vsum = (x[bi].astype(np.float64) @ wv.T.astype(np.float64)).sum(axis=0)
                # the +b term bypasses the g scale, so use the raw w_out
                corr += bb[h] * (w_out[:, h * D:(h + 1) * D].astype(np.float64) @ vsum)
            out[bi] += corr.astype(np.float32)[None, :]

    return out


# revision 20
# speedup vs baseline: 1.2103x; 1.2103x over previous
"""Trainium2 Bass kernel for the L2-normalized attention module.

Reference computation (per batch b):
    qkv = x @ w_qkv.T                        # [n, 3*dim]
    q,k,v per head h (d=64)                  # [n, d]
    dots = q @ k.T                           # [n, n]
    attn = dots / max(||dots_row||_2, eps) * g + b
    out_h = attn @ v                         # [n, d]
    final = concat_h(out_h) @ w_out.T + b_out

Key algebraic factorization: the score "nonlinearity" is only a per-row
scale r_i = 1/max(||dots_i||, eps), and ||dots_i||^2 = q_i^T (k^T k) q_i.
With W = k^T v, G = k^T k:
    out_h^T[:, i] = r_i * (W^T q_i),   r_i = rsqrt(q_i^T G q_i)
so the n x n score matrix never exists.  Additionally W^T is fused into
the output projection on device (F = W^T-blockdiag @ wo^T), so the final
projection consumes rinv-scaled q directly:
    final^T = F^T (q * rinv_broadcast)

Schedule (single PE stream, emission order = execution order):
  stage 1: kv projection + incremental Gram accumulation, chunk by chunk
           (overlapped with the x DMA fill; junk warmup matmuls run
           during the fill to open the HAM clock-gate early).
  stage 2: per 512-column chunk, software-pipelined so the PE never
           idles: q projection of chunk c+1 covers the
           norm2 -> sqrt -> reciprocal -> scale chain of chunk c, and the
           output projection of chunk c covers the next chain.

Sharding: 8 cores = 2 batches x 4 head-groups (4 heads each).  The host
sums the 4 partial w_out products per batch and adds b_out.  norm_g and
the 2^-4 rinv compensation are folded into w_out on the host; norm_b
(zero in practice) is handled by an exact host-side rank-1 correction.
"""

import numpy as np

from concourse import bacc
from concourse import masks
import concourse.mybir as mybir
import concourse.tile as tile
from concourse.bass_utils import run_bass_kernel_spmd

# Problem shape (hardcoded per contract)
B, N, DIM, H, D = 2, 2048, 1024, 16, 64
NCORES = 8
HPC = H // 4            # 4 heads per core
CH = 512                # sequence chunk (matmul moving free dim)
NCH = N // CH           # 4
KO = DIM // 128         # 8 contraction tiles for the projections
P = 128

SG = 2.0 ** -8          # G scale so q*(G q) products stay in fp16 range
SW = 2.0 ** -4          # folded into w_out on host; compensates rinv*2^4

F32 = mybir.dt.float32
F16 = mybir.dt.float16
MULT = mybir.AluOpType.mult
ADD = mybir.AluOpType.add
AFT = mybir.ActivationFunctionType

N_WARM = 64             # junk matmuls bridging the DMA fill


def _build_bass():
    nc = bacc.Bacc("TRN2", target_bir_lowering=False, debug=False)

    # eps^2 const for the sqrt bias (mirrors Bass's const registration)
    _eps_t = nc.alloc_sbuf_tensor("const-float32-eps2q", [128, 1], F32)
    nc.gpsimd.memset(_eps_t.ap(), 2.5e-25)
    nc.const_aps.aps[(F32, 2.5e-25)] = _eps_t.ap()

    x_d = nc.dram_tensor("xt", [NCH, P, KO, CH], F16, kind="ExternalInput").ap()
    wq_d = nc.dram_tensor("wq", [P, KO, 256], F16, kind="ExternalInput").ap()
    wkv_d = nc.dram_tensor("wkv", [P, KO, 512], F16, kind="ExternalInput").ap()
    wo_d = nc.dram_tensor("wo", [P, 2, 1024], F16, kind="ExternalInput").ap()
    out_d = nc.dram_tensor("outT", [NCH, 2, P, 4, CH], F16,
                           kind="ExternalOutput").ap()

    with tile.TileContext(nc) as tc:
        with (
            tc.tile_pool(name="w", bufs=1) as wpool,
            tc.tile_pool(name="big", bufs=1) as bigpool,
            tc.tile_pool(name="small", bufs=4) as smallpool,
            tc.tile_pool(name="stage", bufs=4) as stagepool,
            tc.tile_pool(name="psA", bufs=2, space="PSUM") as psA,
            tc.tile_pool(name="psQ", bufs=2, space="PSUM") as psQ,
            tc.tile_pool(name="psT", bufs=2, space="PSUM") as psT,
            tc.tile_pool(name="psG", bufs=2, space="PSUM") as psG,
        ):
            # ---- on-chip constants (no DMA needed) ----
            warm = wpool.tile([P, 128], F16, name="warm")
            nc.gpsimd.memset(warm, 0.007812)
            bo = wpool.tile([P, 128], F16, name="bo_sb")
            nc.gpsimd.memset(bo, 0.0)
            nc.gpsimd.memset(bo[0:64, 0:64], 1.0)
            nc.gpsimd.memset(bo[64:128, 64:128], 1.0)
            ident = wpool.tile([P, 128], F32, name="ident")
            masks.make_identity(nc, ident)
            gwG_all = wpool.tile([P, 2, 128], F16, name="gwG_all")
            nc.gpsimd.memset(gwG_all, 0.0)
            gwWT_all = wpool.tile([P, 2, 128], F16, name="gwWT_all")
            nc.gpsimd.memset(gwWT_all, 0.0)

            # ---- warmup: opens the HAM clock-gate (K=8/8) during the
            # DMA fill; results are discarded.
            warm_sb = smallpool.tile([P, 4], F32, name="warm_sb")
            ps_warm = psA.tile([P, CH], F32, tag="psA", name="ps_warm")
            for _ in range(N_WARM):
                nc.tensor.matmul(ps_warm[:, 0:128], lhsT=warm, rhs=warm,
                                 start=True, stop=True)
            nc.vector.tensor_copy(warm_sb, ps_warm[:, 0:4])

            # ---- input DMAs, ordered by first use; wkv/x0 split in ko
            # halves so the first kv accumulation group starts ~2.6us
            # earlier than a monolithic transfer would allow.
            wkv = wpool.tile([P, KO, 512], F16, name="wkv_sb")
            x_all = wpool.tile([P, NCH, KO, CH], F16, name="x_all")
            for qq in range(2):
                ks = slice(4 * qq, 4 * qq + 4)
                nc.sync.dma_start(wkv[:, ks, :], wkv_d[:, ks, :])
                nc.sync.dma_start(x_all[:, 0, ks, :], x_d[0, :, ks, :])
            for cc in range(1, NCH):
                nc.sync.dma_start(x_all[:, cc, :, :], x_d[cc])
            wq = wpool.tile([P, KO, 256], F16, name="wq_sb")
            nc.sync.dma_start(wq, wq_d)
            wo = wpool.tile([P, 2, 1024], F16, name="wo_sb")
            nc.sync.dma_start(wo, wo_d)

            # ---- persistent tiles ----
            qT = bigpool.tile([P, 2, N], F16, name="qT_sb")        # [pair, n]
            qs = bigpool.tile([P, 2, N], F16, name="qs_sb")        # rinv-scaled q
            kv = bigpool.tile([P, 16, CH], F16, name="kv_sb")      # [n-tile, per-pair [ke|ko|ve|vo]]
            F_sb = bigpool.tile([P, 2, 1024], F16, name="F_sb")    # fused W^T @ woT
            gwacc = wpool.tile([P, 2, 256], F32, name="gwacc")

            # ---- stage 1: kv projection + incremental Gram ----
            # kv[j, c]  = sum_k x^T[k, j] wkv[k, c]   (c = per-pair [k|v])
            # [G|W] per pair accumulates per kv tile as it is produced.
            # Gram matmuls run one kv tile behind the projection so the PE
            # never waits on the PSUM->SBUF copy of the tile it grams.
            gw_part = [
                psG.tile([P, 256], F32, tag="psG", name=f"gw_part{p}")
                for p in range(2)
            ]

            def gram(jt):
                for p in range(2):
                    blk = kv[:, jt, p * 256:(p + 1) * 256]
                    nc.tensor.matmul(
                        gw_part[p], lhsT=blk[:, 0:128], rhs=blk,
                        start=(jt == 0), stop=(jt == 15),
                    )

            for c in range(NCH):
                for nt in range(4):
                    jt = c * 4 + nt
                    ps_kv = psA.tile([P, CH], F32, tag="psA", name="ps_kv")
                    for ko in range(KO):
                        nc.tensor.matmul(
                            ps_kv,
                            lhsT=(x_all[:, c, ko, nt * 128:(nt + 1) * 128]),
                            rhs=(wkv[:, ko, :]),
                            start=(ko == 0),
                            stop=(ko == KO - 1),
                        )
                    if nt % 2 == 0:
                        nc.scalar.copy(kv[:, jt, :], ps_kv)
                    else:
                        nc.vector.tensor_copy(kv[:, jt, :], ps_kv)
                    if jt > 0:
                        gram(jt - 1)
            gram(15)
            for p in range(2):
                nc.vector.tensor_copy(gwacc[:, p, :], gw_part[p])

            # Gram finalize: block-diagonal G (scaled by SG, two heads
            # stacked on K=128) on DVE/ACT; these overlap the q0 matmuls.
            for p in range(2):
                nc.vector.tensor_scalar_mul(gwG_all[0:64, p, 0:64],
                                            gwacc[0:64, p, 0:64], SG)
                nc.scalar.activation(gwG_all[64:128, p, 64:128],
                                     gwacc[64:128, p, 64:128],
                                     AFT.Copy, bias=0.0, scale=SG)

            # ---- stage 2: q projection pipelined with the norm chain
            # and the fused output projection.
            def q_proj(c):
                cs = slice(c * CH, (c + 1) * CH)
                for mt in range(2):
                    ps_q = psQ.tile([P, CH], F32, tag="psQ", name="ps_q")
                    for ko in range(KO):
                        nc.tensor.matmul(
                            ps_q,
                            lhsT=(wq[:, ko, mt * 128:(mt + 1) * 128]),
                            rhs=(x_all[:, c, ko, :]),
                            start=(ko == 0),
                            stop=(ko == KO - 1),
                        )
                    if mt == 0:
                        nc.vector.tensor_copy(qT[:, mt, cs], ps_q)
                    else:
                        nc.scalar.copy(qT[:, mt, cs], ps_q)

            def t_mm(c):
                # ps_t = G q (PE), prods = (G q) * q elementwise (DVE)
                cs = slice(c * CH, (c + 1) * CH)
                out = {}
                for p in range(2):
                    ps_t = psT.tile([P, CH], F32, tag="psT", name="ps_t")
                    nc.tensor.matmul(ps_t, lhsT=(gwG_all[:, p, :]),
                                     rhs=(qT[:, p, cs]), start=True, stop=True)
                    pr = stagepool.tile([P, CH], F16, name="prod",
                                        tag="prod", bufs=4)
                    nc.vector.tensor_tensor(pr, ps_t, qT[:, p, cs], MULT)
                    out[p] = pr
                return out

            def rep_mm(c, prods):
                # norm2 broadcast per head (fp16 block-ones matmul), then
                # rinv = 1/sqrt(norm2+eps') via ACT Sqrt + DVE reciprocal.
                cs = slice(c * CH, (c + 1) * CH)
                for p in range(2):
                    ps_rep = psT.tile([P, CH], F32, tag="psT", name="ps_rep")
                    nc.tensor.matmul(ps_rep, lhsT=(bo), rhs=(prods[p]),
                                     start=True, stop=True)
                    sq = stagepool.tile([P, CH], F32, name="sqr",
                                        tag="sqr", bufs=4)
                    nc.scalar.activation(sq, ps_rep, AFT.Sqrt, bias=2.5e-25)
                    ri = stagepool.tile([P, CH], F32, name="rinv",
                                        tag="rinv", bufs=4)
                    nc.vector.reciprocal_approx_fast(out=ri, in_=sq)
                    nc.gpsimd.tensor_tensor(qs[:, p, cs], qT[:, p, cs],
                                            ri, MULT)

            def c_proj(c, mts, fine=False, pools=None):
                # final^T partial: F^T (q*rinv), batched stores per kk-half
                # (per-mt stores for the last chunk so the tail is short).
                # pools: psum pools to rotate ps_f through -- the tail C
                # blocks run with q/t/rep retired, so their banks are free.
                if pools is None:
                    pools = [(psA, "psA")]
                cs = slice(c * CH, (c + 1) * CH)
                for kk, st in mts:
                    for mtl in range(4):
                        mt = kk * 4 + mtl
                        pool, ptag = pools[mt % len(pools)]
                        ps_f = pool.tile([P, CH], F32, tag=ptag, name="ps_f")
                        for p in range(2):
                            nc.tensor.matmul(
                                ps_f,
                                lhsT=(F_sb[:, p, mt * 128:(mt + 1) * 128]),
                                rhs=(qs[:, p, cs]),
                                start=(p == 0),
                                stop=(p == 1),
                            )
                        if mtl % 2 == 0:
                            nc.vector.tensor_copy(st[:, mtl, :], ps_f)
                        else:
                            nc.scalar.copy(st[:, mtl, :], ps_f)
                        if fine:
                            nc.sync.dma_start(out_d[c, kk, :, mtl, :],
                                              st[:, mtl, :])
                    if not fine:
                        nc.sync.dma_start(out_d[c, kk], st)

            def st_tiles():
                return [(kk, stagepool.tile([P, 4, CH], F16, name="st",
                                            tag="st", bufs=4))
                        for kk in range(2)]

            # q0, then W^T transposes + F build (covered by q0), then the
            # per-chunk pipeline.
            q_proj(0)

            for p in range(2):
                ps_wt = psG.tile([P, 256], F32, tag="psG", name="ps_wt")
                nc.tensor.transpose(ps_wt[:, 0:128], gwacc[:, p, 128:256],
                                    ident)
                nc.vector.tensor_copy(gwWT_all[0:64, p, 0:64],
                                      ps_wt[0:64, 0:64])
                nc.vector.tensor_copy(gwWT_all[64:128, p, 64:128],
                                      ps_wt[64:128, 64:128])

            t0 = t_mm(0)
            for p in range(2):
                for hf in range(2):
                    ps_F = psA.tile([P, CH], F32, tag="psA", name="ps_F")
                    nc.tensor.matmul(
                        ps_F, lhsT=(gwWT_all[:, p, :]),
                        rhs=(wo[:, p, hf * 512:(hf + 1) * 512]),
                        start=True, stop=True)
                    if hf == 0:
                        nc.scalar.copy(F_sb[:, p, 0:512], ps_F)
                    else:
                        nc.vector.tensor_copy(F_sb[:, p, 512:1024], ps_F)
            rep_mm(0, t0)

            # Chunk 0's output projection is deferred to the very end: its
            # qs is ready early, so it gives the PE chain-free work that
            # covers chunk 3's norm chain (no tail stall).
            q_proj(1)
            t1 = t_mm(1)
            rep_mm(1, t1)

            q_proj(2)
            st1 = st_tiles()
            c_proj(1, st1[:1])
            t2 = t_mm(2)
            c_proj(1, st1[1:])
            rep_mm(2, t2)

            q_proj(3)
            st2 = st_tiles()
            c_proj(2, st2[:1])
            t3 = t_mm(3)
            c_proj(2, st2[1:])
            rep_mm(3, t3)

            endpools = [(psA, "psA"), (psQ, "psQ"), (psT, "psT")]
            c_proj(0, st_tiles(), pools=endpools)
            c_proj(3, st_tiles(), fine=True, pools=endpools)

    nc.compile()
    return nc


_NC_CACHE = None


def _get_nc():
    global _NC_CACHE
    if _NC_CACHE is None:
        _NC_CACHE = _build_bass()
    return _NC_CACHE


def _build_in_maps(x, w_qkv, w_out_g):
    """Per-core device inputs (shared NEFF, different shards)."""
    in_maps = []
    for core in range(NCORES):
        bi = core // 4
        hg = core % 4
        # x^T tiled chunk-major [nch, p, ko, ch]
        xt0 = x[bi].T.reshape(KO, P, N).transpose(1, 0, 2)
        xt = np.ascontiguousarray(
            np.stack([xt0[:, :, cc * CH:(cc + 1) * CH] for cc in range(NCH)]))
        # q rows of this head group, transposed -> [dim, 256] -> [p, ko, 256]
        rows_q = slice(hg * 256, hg * 256 + 256)
        wq = np.ascontiguousarray(
            w_qkv[rows_q].T.reshape(KO, P, 256).transpose(1, 0, 2))
        # per-head-pair [k_even | k_odd | v_even | v_odd] blocks
        blocks = []
        for pp in range(2):
            he = hg * HPC + 2 * pp
            ho = he + 1
            blocks.append(w_qkv[DIM + he * D: DIM + (he + 1) * D])
            blocks.append(w_qkv[DIM + ho * D: DIM + (ho + 1) * D])
            blocks.append(w_qkv[2 * DIM + he * D: 2 * DIM + (he + 1) * D])
            blocks.append(w_qkv[2 * DIM + ho * D: 2 * DIM + (ho + 1) * D])
        wkv_local = np.concatenate(blocks, axis=0)  # [512, dim]
        wkv = np.ascontiguousarray(
            wkv_local.T.reshape(KO, P, 512).transpose(1, 0, 2))
        # w_out columns for this head group (norm_g and SW folded), transposed
        wo_local = w_out_g[:, hg * 256:(hg + 1) * 256]  # [1024, 256]
        wo = np.ascontiguousarray(
            wo_local.T.reshape(2, P, 1024).transpose(1, 0, 2))
        in_maps.append({
            "xt": xt.astype(np.float16), "wq": wq.astype(np.float16),
            "wkv": wkv.astype(np.float16), "wo": wo.astype(np.float16),
        })
    return in_maps


def kernel(x, w_qkv, w_out, b_out, norm_g, norm_b):
    x = np.ascontiguousarray(np.asarray(x, dtype=np.float32))
    w_qkv = np.asarray(w_qkv, dtype=np.float32)
    w_out = np.asarray(w_out, dtype=np.float32)
    b_out = np.asarray(b_out, dtype=np.float32)
    g = np.asarray(norm_g, dtype=np.float32).reshape(H)
    bb = np.asarray(norm_b, dtype=np.float32).reshape(H)

    # Fold norm_g and the 2^-4 rinv compensation into w_out columns.
    w_out_g = w_out.copy()
    for h in range(H):
        w_out_g[:, h * D:(h + 1) * D] *= g[h] * SW

    in_maps = _build_in_maps(x, w_qkv, w_out_g)

    nc = _get_nc()
    res = None
    last_exc = None
    for _attempt in range(3):
        try:
            res = run_bass_kernel_spmd(nc, in_maps, core_ids=list(range(NCORES)))
            break
        except Exception as e:  # transient NRT_EXEC_UNIT_UNRECOVERABLE etc.
            last_exc = e
            import time as _time
            _time.sleep(5)
    if res is None:
        raise last_exc

    out = np.zeros((B, N, DIM), np.float32)
    for core in range(NCORES):
        bi = core // 4
        buf = res.results[core]["outT"]  # [NCH, 2, P, 4, CH]
        partial = buf.transpose(1, 3, 2, 0, 4).reshape(DIM, N).astype(np.float32)
        out[bi] += partial.T
    out += b_out[None, None, :]

    # Exact rank-1 correction for norm_b (zero in practice).
    if np.any(bb != 0.0):
        for bi in range(B):
            corr = np.zeros(DIM, np.float64)
            for h in range(H):
                wv = w_qkv[2 * DIM + h * D: 2 * DIM + (h + 1) * D]  # [d, dim]
                vsum = (x[bi].astype(np.float64) @ wv.T.astype(np.float64)).sum(axis=0)
                # the +b term bypasses the g scale, so use the raw w_out
                corr += bb[h] * (w_out[:, h * D:(h + 1) * D].astype(np.float64) @ vsum)
            out[bi] += corr.astype(np.float32)[None, :]

    return out


# revision 23
# speedup vs baseline: 1.2319x; 1.0178x over previous
"""Trainium2 Bass kernel for the L2-normalized attention module.

Reference computation (per batch b):
    qkv = x @ w_qkv.T                        # [n, 3*dim]
    q,k,v per head h (d=64)                  # [n, d]
    dots = q @ k.T                           # [n, n]
    attn = dots / max(||dots_row||_2, eps) * g + b
    out_h = attn @ v                         # [n, d]
    final = concat_h(out_h) @ w_out.T + b_out

Key algebraic factorization: the score "nonlinearity" is only a per-row
scale r_i = 1/max(||dots_i||, eps), and ||dots_i||^2 = q_i^T (k^T k) q_i.
With W = k^T v, G = k^T k:
    out_h^T[:, i] = r_i * (W^T q_i),   r_i = rsqrt(q_i^T G q_i)
so the n x n score matrix never exists.  Additionally W^T is fused into
the output projection on device (F = W^T-blockdiag @ wo^T), so the final
projection consumes rinv-scaled q directly:
    final^T = F^T (q * rinv_broadcast)

Schedule (single PE stream, emission order = execution order):
  stage 1: kv projection + incremental Gram accumulation, chunk by chunk
           (overlapped with the x DMA fill; junk warmup matmuls run
           during the fill to open the HAM clock-gate early).
  stage 2: per 512-column chunk, software-pipelined so the PE never
           idles: q projection of chunk c+1 covers the
           norm2 -> sqrt -> reciprocal -> scale chain of chunk c, and the
           output projection of chunk c covers the next chain.

Sharding: 8 cores = 2 batches x 4 head-groups (4 heads each).  The host
sums the 4 partial w_out products per batch and adds b_out.  norm_g and
the 2^-4 rinv compensation are folded into w_out on the host; norm_b
(zero in practice) is handled by an exact host-side rank-1 correction.
"""

import numpy as np

from concourse import bacc
from concourse import masks
import concourse.mybir as mybir
import concourse.tile as tile
from concourse.bass_utils import run_bass_kernel_spmd

# Problem shape (hardcoded per contract)
B, N, DIM, H, D = 2, 2048, 1024, 16, 64
NCORES = 8
HPC = H // 4            # 4 heads per core
CH = 512                # sequence chunk (matmul moving free dim)
NCH = N // CH           # 4
KO = DIM // 128         # 8 contraction tiles for the projections
P = 128

SG = 2.0 ** -8          # G scale so q*(G q) products stay in fp16 range
SW = 2.0 ** -4          # folded into w_out on host; compensates rinv*2^4

F32 = mybir.dt.float32
F16 = mybir.dt.float16
MULT = mybir.AluOpType.mult
ADD = mybir.AluOpType.add
AFT = mybir.ActivationFunctionType

N_WARM = 72             # junk matmuls bridging the DMA fill
N_WARM2 = 12            # filler between the first kv halves (x0b wait)


def _build_bass():
    nc = bacc.Bacc("TRN2", target_bir_lowering=False, debug=False)

    # eps^2 const for the sqrt bias (mirrors Bass's const registration)
    _eps_t = nc.alloc_sbuf_tensor("const-float32-eps2q", [128, 1], F32)
    nc.gpsimd.memset(_eps_t.ap(), 2.5e-25)
    nc.const_aps.aps[(F32, 2.5e-25)] = _eps_t.ap()

    x_d = nc.dram_tensor("xt", [NCH, P, KO, CH], F16, kind="ExternalInput").ap()
    wq_d = nc.dram_tensor("wq", [P, KO, 256], F16, kind="ExternalInput").ap()
    wkv_d = nc.dram_tensor("wkv", [P, KO, 512], F16, kind="ExternalInput").ap()
    wo_d = nc.dram_tensor("wo", [P, 2, 1024], F16, kind="ExternalInput").ap()
    out_d = nc.dram_tensor("outT", [NCH, 2, P, 4, CH], F16,
                           kind="ExternalOutput").ap()

    with tile.TileContext(nc) as tc:
        with (
            tc.tile_pool(name="w", bufs=1) as wpool,
            tc.tile_pool(name="big", bufs=1) as bigpool,
            tc.tile_pool(name="small", bufs=4) as smallpool,
            tc.tile_pool(name="stage", bufs=4) as stagepool,
            tc.tile_pool(name="psA", bufs=2, space="PSUM") as psA,
            tc.tile_pool(name="psQ", bufs=2, space="PSUM") as psQ,
            tc.tile_pool(name="psT", bufs=2, space="PSUM") as psT,
            tc.tile_pool(name="psG", bufs=2, space="PSUM") as psG,
        ):
            # ---- on-chip constants (no DMA needed) ----
            warm = wpool.tile([P, 128], F16, name="warm")
            nc.gpsimd.memset(warm, 0.007812)
            bo = wpool.tile([P, 128], F16, name="bo_sb")
            nc.gpsimd.memset(bo, 0.0)
            nc.gpsimd.memset(bo[0:64, 0:64], 1.0)
            nc.gpsimd.memset(bo[64:128, 64:128], 1.0)
            ident = wpool.tile([P, 128], F32, name="ident")
            masks.make_identity(nc, ident)
            gwG_all = wpool.tile([P, 2, 128], F16, name="gwG_all")
            nc.gpsimd.memset(gwG_all, 0.0)
            gwWT_all = wpool.tile([P, 2, 128], F16, name="gwWT_all")
            nc.gpsimd.memset(gwWT_all, 0.0)

            # ---- warmup: opens the HAM clock-gate (K=8/8) during the
            # DMA fill; results are discarded.
            warm_sb = smallpool.tile([P, 4], F32, name="warm_sb")
            ps_warm = psA.tile([P, CH], F32, tag="psA", name="ps_warm")
            for _ in range(N_WARM):
                nc.tensor.matmul(ps_warm[:, 0:128], lhsT=warm, rhs=warm,
                                 start=True, stop=True)
            nc.vector.tensor_copy(warm_sb, ps_warm[:, 0:4])

            # ---- input DMAs, ordered by first use; wkv/x0 split in ko
            # halves so the first kv accumulation group starts ~2.6us
            # earlier than a monolithic transfer would allow.
            wkv = wpool.tile([P, KO, 512], F16, name="wkv_sb")
            x_all = wpool.tile([P, NCH, KO, CH], F16, name="x_all")
            for qq in range(2):
                ks = slice(4 * qq, 4 * qq + 4)
                nc.sync.dma_start(wkv[:, ks, :], wkv_d[:, ks, :])
                nc.sync.dma_start(x_all[:, 0, ks, :], x_d[0, :, ks, :])
            for cc in range(1, NCH):
                nc.sync.dma_start(x_all[:, cc, :, :], x_d[cc])
            wq = wpool.tile([P, KO, 256], F16, name="wq_sb")
            nc.sync.dma_start(wq, wq_d)
            wo = wpool.tile([P, 2, 1024], F16, name="wo_sb")
            nc.sync.dma_start(wo, wo_d)

            # ---- persistent tiles ----
            qT = bigpool.tile([P, 2, N], F16, name="qT_sb")        # [pair, n]
            qs = bigpool.tile([P, 2, N], F16, name="qs_sb")        # rinv-scaled q
            kv = bigpool.tile([P, 16, CH], F16, name="kv_sb")      # [n-tile, per-pair [ke|ko|ve|vo]]
            F_sb = bigpool.tile([P, 2, 1024], F16, name="F_sb")    # fused W^T @ woT
            gwacc = wpool.tile([P, 2, 256], F32, name="gwacc")

            # ---- stage 1: kv projection + incremental Gram ----
            # kv[j, c]  = sum_k x^T[k, j] wkv[k, c]   (c = per-pair [k|v])
            # [G|W] per pair accumulates per kv tile as it is produced.
            # Gram matmuls run one kv tile behind the projection so the PE
            # never waits on the PSUM->SBUF copy of the tile it grams.
            gw_part = [
                psG.tile([P, 256], F32, tag="psG", name=f"gw_part{p}")
                for p in range(2)
            ]

            def gram(jt):
                for p in range(2):
                    blk = kv[:, jt, p * 256:(p + 1) * 256]
                    nc.tensor.matmul(
                        gw_part[p], lhsT=blk[:, 0:128], rhs=blk,
                        start=(jt == 0), stop=(jt == 15),
                    )

            for c in range(NCH):
                for nt in range(4):
                    jt = c * 4 + nt
                    ps_kv = psA.tile([P, CH], F32, tag="psA", name="ps_kv")
                    for ko in range(KO):
                        nc.tensor.matmul(
                            ps_kv,
                            lhsT=(x_all[:, c, ko, nt * 128:(nt + 1) * 128]),
                            rhs=(wkv[:, ko, :]),
                            start=(ko == 0),
                            stop=(ko == KO - 1),
                        )
                        if jt == 0 and ko == 3:
                            # filler while the second x0 half lands
                            for _ in range(N_WARM2):
                                nc.tensor.matmul(ps_warm[:, 0:128],
                                                 lhsT=warm, rhs=warm,
                                                 start=True, stop=True)
                    if nt % 2 == 0:
                        nc.scalar.copy(kv[:, jt, :], ps_kv)
                    else:
                        nc.vector.tensor_copy(kv[:, jt, :], ps_kv)
                    if jt > 1:
                        gram(jt - 2)
            gram(14)
            gram(15)
            for p in range(2):
                nc.vector.tensor_copy(gwacc[:, p, :], gw_part[p])

            # Gram finalize: block-diagonal G (scaled by SG, two heads
            # stacked on K=128) on DVE/ACT; these overlap the q0 matmuls.
            for p in range(2):
                nc.vector.tensor_scalar_mul(gwG_all[0:64, p, 0:64],
                                            gwacc[0:64, p, 0:64], SG)
                nc.scalar.activation(gwG_all[64:128, p, 64:128],
                                     gwacc[64:128, p, 64:128],
                                     AFT.Copy, bias=0.0, scale=SG)

            # ---- stage 2: q projection pipelined with the norm chain
            # and the fused output projection.
            def q_proj(c):
                cs = slice(c * CH, (c + 1) * CH)
                for mt in range(2):
                    ps_q = psQ.tile([P, CH], F32, tag="psQ", name="ps_q")
                    for ko in range(KO):
                        nc.tensor.matmul(
                            ps_q,
                            lhsT=(wq[:, ko, mt * 128:(mt + 1) * 128]),
                            rhs=(x_all[:, c, ko, :]),
                            start=(ko == 0),
                            stop=(ko == KO - 1),
                        )
                    if mt == 0:
                        nc.vector.tensor_copy(qT[:, mt, cs], ps_q)
                    else:
                        nc.scalar.copy(qT[:, mt, cs], ps_q)

            def t_mm(c):
                # ps_t = G q (PE), prods = (G q) * q elementwise (DVE)
                cs = slice(c * CH, (c + 1) * CH)
                out = {}
                for p in range(2):
                    ps_t = psT.tile([P, CH], F32, tag="psT", name="ps_t")
                    nc.tensor.matmul(ps_t, lhsT=(gwG_all[:, p, :]),
                                     rhs=(qT[:, p, cs]), start=True, stop=True)
                    pr = stagepool.tile([P, CH], F16, name="prod",
                                        tag="prod", bufs=4)
                    nc.vector.tensor_tensor(pr, ps_t, qT[:, p, cs], MULT)
                    out[p] = pr
                return out

            def rep_mm(c, prods):
                # norm2 broadcast per head (fp16 block-ones matmul), then
                # rinv = 1/sqrt(norm2+eps') via ACT Sqrt + DVE reciprocal.
                cs = slice(c * CH, (c + 1) * CH)
                for p in range(2):
                    ps_rep = psT.tile([P, CH], F32, tag="psT", name="ps_rep")
                    nc.tensor.matmul(ps_rep, lhsT=(bo), rhs=(prods[p]),
                                     start=True, stop=True)
                    sq = stagepool.tile([P, CH], F32, name="sqr",
                                        tag="sqr", bufs=4)
                    nc.scalar.activation(sq, ps_rep, AFT.Sqrt, bias=2.5e-25)
                    ri = stagepool.tile([P, CH], F32, name="rinv",
                                        tag="rinv", bufs=4)
                    nc.vector.reciprocal_approx_fast(out=ri, in_=sq)
                    nc.gpsimd.tensor_tensor(qs[:, p, cs], qT[:, p, cs],
                                            ri, MULT)

            def c_proj(c, mts, fine=False, pools=None):
                # final^T partial: F^T (q*rinv), batched stores per kk-half
                # (per-mt stores for the last chunk so the tail is short).
                # pools: psum pools to rotate ps_f through -- the tail C
                # blocks run with q/t/rep retired, so their banks are free.
                if pools is None:
                    pools = [(psA, "psA")]
                cs = slice(c * CH, (c + 1) * CH)
                for kk, st in mts:
                    for mtl in range(4):
                        mt = kk * 4 + mtl
                        pool, ptag = pools[mt % len(pools)]
                        ps_f = pool.tile([P, CH], F32, tag=ptag, name="ps_f")
                        for p in range(2):
                            nc.tensor.matmul(
                                ps_f,
                                lhsT=(F_sb[:, p, mt * 128:(mt + 1) * 128]),
                                rhs=(qs[:, p, cs]),
                                start=(p == 0),
                                stop=(p == 1),
                            )
                        if mtl % 2 == 0:
                            nc.vector.tensor_copy(st[:, mtl, :], ps_f)
                        else:
                            nc.scalar.copy(st[:, mtl, :], ps_f)
                        if fine and mtl % 2 == 1:
                            nc.sync.dma_start(out_d[c, kk, :, mtl - 1:mtl + 1, :],
                                              st[:, mtl - 1:mtl + 1, :])
                    if not fine:
                        nc.sync.dma_start(out_d[c, kk], st)

            def st_tiles():
                return [(kk, stagepool.tile([P, 4, CH], F16, name="st",
                                            tag="st", bufs=4))
                        for kk in range(2)]

            # q0, then W^T transposes + F build (covered by q0), then the
            # per-chunk pipeline.
            q_proj(0)

            for p in range(2):
                ps_wt = psG.tile([P, 256], F32, tag="psG", name="ps_wt")
                nc.tensor.transpose(ps_wt[:, 0:128], gwacc[:, p, 128:256],
                                    ident)
                nc.vector.tensor_copy(gwWT_all[0:64, p, 0:64],
                                      ps_wt[0:64, 0:64])
                nc.vector.tensor_copy(gwWT_all[64:128, p, 64:128],
                                      ps_wt[64:128, 64:128])

            t0 = t_mm(0)
            for p in range(2):
                for hf in range(2):
                    ps_F = psA.tile([P, CH], F32, tag="psA", name="ps_F")
                    nc.tensor.matmul(
                        ps_F, lhsT=(gwWT_all[:, p, :]),
                        rhs=(wo[:, p, hf * 512:(hf + 1) * 512]),
                        start=True, stop=True)
                    if hf == 0:
                        nc.scalar.copy(F_sb[:, p, 0:512], ps_F)
                    else:
                        nc.vector.tensor_copy(F_sb[:, p, 512:1024], ps_F)
            rep_mm(0, t0)

            # Chunk 0's output projection is deferred to the very end: its
            # qs is ready early, so it gives the PE chain-free work that
            # covers chunk 3's norm chain (no tail stall).
            q_proj(1)
            t1 = t_mm(1)
            rep_mm(1, t1)

            q_proj(2)
            st1 = st_tiles()
            c_proj(1, st1[:1])
            t2 = t_mm(2)
            c_proj(1, st1[1:])
            rep_mm(2, t2)

            q_proj(3)
            st2 = st_tiles()
            c_proj(2, st2[:1])
            t3 = t_mm(3)
            c_proj(2, st2[1:])
            rep_mm(3, t3)

            endpools = [(psA, "psA"), (psQ, "psQ"), (psT, "psT")]
            c_proj(0, st_tiles(), pools=endpools)
            c_proj(3, st_tiles(), fine=True, pools=endpools)

    nc.compile()
    return nc


_NC_CACHE = None


def _get_nc():
    global _NC_CACHE
    if _NC_CACHE is None:
        _NC_CACHE = _build_bass()
    return _NC_CACHE


def _build_in_maps(x, w_qkv, w_out_g):
    """Per-core device inputs (shared NEFF, different shards)."""
    in_maps = []
    for core in range(NCORES):
        bi = core // 4
        hg = core % 4
        # x^T tiled chunk-major [nch, p, ko, ch]
        xt0 = x[bi].T.reshape(KO, P, N).transpose(1, 0, 2)
        xt = np.ascontiguousarray(
            np.stack([xt0[:, :, cc * CH:(cc + 1) * CH] for cc in range(NCH)]))
        # q rows of this head group, transposed -> [dim, 256] -> [p, ko, 256]
        rows_q = slice(hg * 256, hg * 256 + 256)
        wq = np.ascontiguousarray(
            w_qkv[rows_q].T.reshape(KO, P, 256).transpose(1, 0, 2))
        # per-head-pair [k_even | k_odd | v_even | v_odd] blocks
        blocks = []
        for pp in range(2):
            he = hg * HPC + 2 * pp
            ho = he + 1
            blocks.append(w_qkv[DIM + he * D: DIM + (he + 1) * D])
            blocks.append(w_qkv[DIM + ho * D: DIM + (ho + 1) * D])
            blocks.append(w_qkv[2 * DIM + he * D: 2 * DIM + (he + 1) * D])
            blocks.append(w_qkv[2 * DIM + ho * D: 2 * DIM + (ho + 1) * D])
        wkv_local = np.concatenate(blocks, axis=0)  # [512, dim]
        wkv = np.ascontiguousarray(
            wkv_local.T.reshape(KO, P, 512).transpose(1, 0, 2))
        # w_out columns for this head group (norm_g and SW folded), transposed
        wo_local = w_out_g[:, hg * 256:(hg + 1) * 256]  # [1024, 256]
        wo = np.ascontiguousarray(
            wo_local.T.reshape(2, P, 1024).transpose(1, 0, 2))
        in_maps.append({
            "xt": xt.astype(np.float16), "wq": wq.astype(np.float16),
            "wkv": wkv.astype(np.float16), "wo": wo.astype(np.float16),
        })
    return in_maps


def kernel(x, w_qkv, w_out, b_out, norm_g, norm_b):
    x = np.ascontiguousarray(np.asarray(x, dtype=np.float32))
    w_qkv = np.asarray(w_qkv, dtype=np.float32)
    w_out = np.asarray(w_out, dtype=np.float32)
    b_out = np.asarray(b_out, dtype=np.float32)
    g = np.asarray(norm_g, dtype=np.float32).reshape(H)
    bb = np.asarray(norm_b, dtype=np.float32).reshape(H)

    # Fold norm_g and the 2^-4 rinv compensation into w_out columns.
    w_out_g = w_out.copy()
    for h in range(H):
        w_out_g[:, h * D:(h + 1) * D] *= g[h] * SW

    in_maps = _build_in_maps(x, w_qkv, w_out_g)

    nc = _get_nc()
    res = None
    last_exc = None
    for _attempt in range(3):
        try:
            res = run_bass_kernel_spmd(nc, in_maps, core_ids=list(range(NCORES)))
            break
        except Exception as e:  # transient NRT_EXEC_UNIT_UNRECOVERABLE etc.
            last_exc = e
            import time as _time
            _time.sleep(5)
    if res is None:
        raise last_exc

    out = np.zeros((B, N, DIM), np.float32)
    for core in range(NCORES):
        bi = core // 4
        buf = res.results[core]["outT"]  # [NCH, 2, P, 4, CH]
        partial = buf.transpose(1, 3, 2, 0, 4).reshape(DIM, N).astype(np.float32)
        out[bi] += partial.T
    out += b_out[None, None, :]

    # Exact rank-1 correction for norm_b (zero in practice).
    if np.any(bb != 0.0):
        for bi in range(B):
            corr = np.zeros(DIM, np.float64)
            for h in range(H):
                wv = w_qkv[2 * DIM + h * D: 2 * DIM + (h + 1) * D]  # [d, dim]
                vsum = (x[bi].astype(np.float64) @ wv.T.astype(np.float64)).sum(axis=0)
                # the +b term bypasses the g scale, so use the raw w_out
                corr += bb[h] * (w_out[:, h * D:(h + 1) * D].astype(np.float64) @ vsum)
            out[bi] += corr.astype(np.float32)[None, :]

    return out
